# revision 7
# baseline (speedup 1.0000x reference)
"""Trainium2 Bass kernel for segment_sum (scatter-add of edge features into nodes).

Strategy: 2M edges split contiguously across 8 NeuronCores (250k each).
Host-side prep (layout only, no FP arithmetic): sort each core's edges by
node id, cut the sorted stream into 128 partition streams at node-run
boundaries, pad each stream to 2048 slots, and build a run-continuation
mask m (m=0 at the first edge of each node run, 1 inside a run).

Device (per core): the whole reduction is a segmented scan on the DVE:
    state = m[t] * state + h[t]        (fp32 internal state)
run per feature channel d (32 contiguous in-place scans per 1024-slot
piece, 2 pieces chained via `initial`; ~2.08 ns/element measured). At
the last slot of each node run, `state` holds that node's complete
per-core sum. The full scan stream is DMA'd back (bf16) and the host
picks the run-end slots and adds the 8 per-core partials (same
unshard-add as the original baseline). Input DMAs are split by feature
group so the first scan starts ~10us in; output DMAs are chunked with a
fine-grained tail so the last transfer is small.

No PE, no GPSIMD, no gather: HBM traffic is 2 x 16.8 MB/core of
contiguous bf16 and the DVE scan runs at ~2 cycles/element.
"""
import numpy as np
import ml_dtypes

import concourse.bacc as bacc
import concourse.mybir as mybir
from concourse import tile
from concourse.bass_utils import run_bass_kernel_spmd

BF16 = mybir.dt.bfloat16
OP = mybir.AluOpType

E = 2_000_000
D = 32
N = 100_000
CORES = 8
EPC = E // CORES            # 250_000
PARTS = 128
SLOTS = 2048                # padded edge slots per partition stream
PIECES = 2
PLEN = SLOTS // PIECES      # 1024 slots per piece
PFREE = D * PLEN            # free elements per piece
FREE = PIECES * PFREE
# input d-groups (first group small so the first scan starts early)
IN_GROUPS = [2, 2, 4, 4, 4, 4, 4, 4, 4]
# output d-groups per piece (last group of the last piece split fine to
# shrink the final-DMA tail)
OUT_GROUPS = [4, 4, 4, 4, 4, 4, 4, 2, 1, 1]


def build_program():
    nc = bacc.Bacc("TRN2", target_bir_lowering=False, debug=False,
                   num_devices=CORES)
    h_in = nc.dram_tensor("h", [PARTS, FREE], BF16, kind="ExternalInput")
    m_in = nc.dram_tensor("m", [PARTS, SLOTS], BF16, kind="ExternalInput")
    s_out = nc.dram_tensor("s", [PARTS, FREE], BF16, kind="ExternalOutput")

    with tile.TileContext(nc) as tc:
        with tc.tile_pool(name="mask", bufs=1) as mp, \
             tc.tile_pool(name="work", bufs=2) as wp:
            mt = mp.tile([PARTS, SLOTS], BF16)
            nc.sync.dma_start(mt[:], m_in[:])
            tiles = []
            # issue every input DMA upfront; both piece buffers exist
            # (bufs=2) so piece 1 streams in while piece 0 scans
            for k in range(PIECES):
                ht = wp.tile([PARTS, PFREE], BF16, tag="h")
                d0 = 0
                for ng in IN_GROUPS:
                    lo = d0 * PLEN
                    hi = (d0 + ng) * PLEN
                    nc.sync.dma_start(
                        ht[:, lo:hi],
                        h_in[:, k * PFREE + lo:k * PFREE + hi])
                    d0 += ng
                tiles.append(ht)
            prev = None
            for k in range(PIECES):
                ht = tiles[k]
                d = 0
                for ng in OUT_GROUPS:
                    for dd in range(d, d + ng):
                        lo = dd * PLEN
                        hi = lo + PLEN
                        init = 0.0 if prev is None else prev[:, hi - 1:hi]
                        # in-place: the scan overwrites the h tile
                        nc.vector.tensor_tensor_scan(
                            ht[:, lo:hi],
                            mt[:, k * PLEN:(k + 1) * PLEN],
                            ht[:, lo:hi],
                            init, OP.mult, OP.add)
                    lo = d * PLEN
                    hi = (d + ng) * PLEN
                    nc.sync.dma_start(
                        s_out[:, k * PFREE + lo:k * PFREE + hi],
                        ht[:, lo:hi])
                    d += ng
                prev = ht
    nc.compile()
    return nc


_prog_cache = {}


def _get_prog():
    if "nc" not in _prog_cache:
        _prog_cache["nc"] = build_program()
    return _prog_cache["nc"]


def kernel(H, X_node, node_num):
    H = np.ascontiguousarray(np.asarray(H, dtype=np.float32))
    X = np.asarray(X_node).astype(np.int64)
    assert H.shape == (E, D) and X.shape == (E,)
    nc = _get_prog()

    in_maps = []
    metas = []
    tgt = np.arange(1, PARTS) * ((EPC + PARTS - 1) // PARTS)
    for c in range(CORES):
        Xc = X[c * EPC:(c + 1) * EPC]
        Hc = H[c * EPC:(c + 1) * EPC]
        perm = np.argsort(Xc, kind="stable")
        Xs = Xc[perm]
        Hs = Hc[perm]
        # node-run starts; cut the stream into 128 partition streams at
        # run boundaries so no node spans two partitions
        runstarts = np.concatenate(
            [[0], np.flatnonzero(np.diff(Xs)) + 1])
        ci = np.searchsorted(runstarts, tgt, side="left")
        ci = np.minimum(ci, len(runstarts) - 1)
        cuts = np.concatenate([[0], runstarts[ci], [EPC]])
        cnt = np.diff(cuts)
        assert cnt.max() <= SLOTS, f"partition stream overflow: {cnt.max()}"

        node_pad = np.full((PARTS, SLOTS), -1, np.int64)
        h_pad = np.zeros((PARTS, SLOTS, D), np.float32)
        pidx = np.repeat(np.arange(PARTS), cnt)
        eidx = np.arange(EPC) - np.repeat(cuts[:-1], cnt)
        node_pad[pidx, eidx] = Xs
        h_pad[pidx, eidx] = Hs
        m = np.zeros((PARTS, SLOTS), np.float32)
        m[:, 1:] = node_pad[:, 1:] == node_pad[:, :-1]

        h_dev = np.ascontiguousarray(
            h_pad.reshape(PARTS, PIECES, PLEN, D).transpose(0, 1, 3, 2)
        ).reshape(PARTS, FREE).astype(ml_dtypes.bfloat16)
        m_dev = m.astype(ml_dtypes.bfloat16)
        in_maps.append({"h": h_dev, "m": np.ascontiguousarray(m_dev)})
        metas.append(node_pad)

    _prog_cache["last_inputs"] = in_maps
    # The very first execution of a freshly loaded program has been
    # observed (once) to return corrupted results; correct runs are
    # bit-identical. Run until two consecutive executions agree.
    res = run_bass_kernel_spmd(nc, in_maps, core_ids=list(range(CORES)),
                               trace=False)
    for _ in range(3):
        res2 = run_bass_kernel_spmd(nc, in_maps, core_ids=list(range(CORES)),
                                    trace=False)
        if all(
            np.array_equal(
                res.results[c]["s"].view(np.uint16),
                res2.results[c]["s"].view(np.uint16))
            for c in range(CORES)
        ):
            break
        res = res2

    out = np.zeros((N, D), np.float32)
    for c in range(CORES):
        node_pad = metas[c]
        s = np.asarray(res.results[c]["s"]).astype(np.float32)
        s = s.reshape(PARTS, PIECES, D, PLEN)
        nxt = np.concatenate(
            [node_pad[:, 1:], np.full((PARTS, 1), -2, np.int64)], axis=1)
        is_end = (node_pad >= 0) & (node_pad != nxt)
        pp, ii = np.nonzero(is_end)
        nodes = node_pad[pp, ii]
        vals = s[pp, ii // PLEN, :, ii % PLEN]
        # within one core each node has exactly one run end -> unique idx
        out[nodes] += vals
    return out


# revision 8
# speedup vs baseline: 1.2201x; 1.2201x over previous
"""Trainium2 Bass kernel for segment_sum (scatter-add of edge features into nodes).

Strategy: 2M edges split contiguously across 8 NeuronCores (250k each).
Host-side prep (layout only, no FP arithmetic): sort each core's edges by
node id, pad every node run to EVEN length (pad slots carry h=0 inside
the run), cut the padded stream into 128 partition streams at run
boundaries, pad each to 2560 slots, and deinterleave each 1280-slot
piece into step-1 A/B halves (A = even slots, B = odd slots of each
pair).

Device (per core), all on the DVE:
  1. Pairing pass: A += B with a plain tensor_tensor add. All operands
     are bf16 step-1, so the DVE runs it in 2x mode (2 el/cycle); each
     pair is two edges of the SAME node (guaranteed by the even-run
     padding), so A becomes the pair-sum stream at half the length.
  2. Segmented scan over A:  state = m2[t]*state + A[t]  (fp32 state,
     ~2.08 ns/el) per feature channel, chained across pieces via
     `initial`. At the last pair of each node run, state holds that
     node's complete per-core sum.
The A halves are DMA'd back (bf16); the host picks the run-end pairs
and adds the 8 per-core partials (same unshard-add as the original
baseline).

vs. the plain-scan version this halves the scan element count for a
~20% padding overhead: DVE busy drops from ~147us to ~125us/core.
"""
import numpy as np
import ml_dtypes

import concourse.bass as bass
import concourse.bacc as bacc
import concourse.mybir as mybir
from concourse import tile
from concourse.bass_utils import run_bass_kernel_spmd

BF16 = mybir.dt.bfloat16
OP = mybir.AluOpType

E = 2_000_000
D = 32
N = 100_000
CORES = 8
EPC = E // CORES            # 250_000
PARTS = 128
SLOTS = 2560                # even-run-padded raw slots per partition
PIECES = 2
PLEN = SLOTS // PIECES      # 1280 raw slots per piece (per channel row)
HLEN = PLEN // 2            # 640 pairs per piece
ROW = D * PLEN              # free elements per piece
FREE = PIECES * ROW
TTG = 4                     # channels per pairing-TT / input-DMA group
# output d-groups per piece (fine tail shrinks the final-DMA drain)
OUT_GROUPS = [4, 4, 4, 4, 4, 4, 4, 2, 1, 1]


def build_program():
    nc = bacc.Bacc("TRN2", target_bir_lowering=False, debug=False,
                   num_devices=CORES)
    h_in = nc.dram_tensor("h", [PARTS, FREE], BF16, kind="ExternalInput")
    m_in = nc.dram_tensor("m", [PARTS, PIECES * HLEN], BF16,
                          kind="ExternalInput")
    s_out = nc.dram_tensor("s", [PARTS, PIECES * D * HLEN], BF16,
                           kind="ExternalOutput")

    with tile.TileContext(nc) as tc:
        with tc.tile_pool(name="mask", bufs=1) as mp, \
             tc.tile_pool(name="work", bufs=2) as wp:
            mt = mp.tile([PARTS, PIECES * HLEN], BF16)
            nc.sync.dma_start(mt[:], m_in[:])
            tiles = []
            # issue every input DMA upfront; both piece buffers exist
            # (bufs=2) so piece 1 streams in while piece 0 computes
            for k in range(PIECES):
                ht = wp.tile([PARTS, ROW], BF16, tag="h")
                for g in range(D // TTG):
                    lo = g * TTG * PLEN
                    hi = (g + 1) * TTG * PLEN
                    nc.sync.dma_start(
                        ht[:, lo:hi],
                        h_in[:, k * ROW + lo:k * ROW + hi])
                tiles.append(ht)
            prev = None
            for k in range(PIECES):
                ht = tiles[k]
                # pairing pass, one in-place 2x TT per TTG channels:
                # A[d, j] += B[d, j]
                for g in range(D // TTG):
                    base = g * TTG * PLEN
                    a_ap = bass.AP(ht.tensor, base,
                                   [[ROW, PARTS], [PLEN, TTG], [1, HLEN]])
                    b_ap = bass.AP(ht.tensor, base + HLEN,
                                   [[ROW, PARTS], [PLEN, TTG], [1, HLEN]])
                    nc.vector.tensor_tensor(a_ap, a_ap, b_ap, OP.add)
                d = 0
                for ng in OUT_GROUPS:
                    for dd in range(d, d + ng):
                        lo = dd * PLEN
                        init = (0.0 if prev is None
                                else prev[:, lo + HLEN - 1:lo + HLEN])
                        # in-place segmented scan over the A half
                        nc.vector.tensor_tensor_scan(
                            ht[:, lo:lo + HLEN],
                            mt[:, k * HLEN:(k + 1) * HLEN],
                            ht[:, lo:lo + HLEN],
                            init, OP.mult, OP.add)
                    src = bass.AP(ht.tensor, d * PLEN,
                                  [[ROW, PARTS], [PLEN, ng], [1, HLEN]])
                    nc.sync.dma_start(
                        s_out[:, (k * D + d) * HLEN:(k * D + d + ng) * HLEN],
                        src)
                    d += ng
                prev = ht
    nc.compile()
    return nc


_prog_cache = {}


def _get_prog():
    if "nc" not in _prog_cache:
        _prog_cache["nc"] = build_program()
    return _prog_cache["nc"]


def kernel(H, X_node, node_num):
    H = np.ascontiguousarray(np.asarray(H, dtype=np.float32))
    X = np.asarray(X_node).astype(np.int64)
    assert H.shape == (E, D) and X.shape == (E,)
    nc = _get_prog()

    in_maps = []
    metas = []
    for c in range(CORES):
        Xc = X[c * EPC:(c + 1) * EPC]
        Hc = H[c * EPC:(c + 1) * EPC]
        perm = np.argsort(Xc, kind="stable")
        Xs = Xc[perm]
        Hs = Hc[perm]
        # node runs; pad each run to even length (pad slot: h=0, same node)
        runstarts = np.concatenate([[0], np.flatnonzero(np.diff(Xs)) + 1])
        R = len(runstarts)
        L = np.diff(np.concatenate([runstarts, [EPC]]))
        odd = (L & 1).astype(bool)
        start2 = np.concatenate([[0], np.cumsum(L + (L & 1))])
        T = int(start2[-1])
        run_of = np.repeat(np.arange(R), L)
        pos2 = start2[:-1][run_of] + (np.arange(EPC) - runstarts[run_of])
        node2 = np.full(T, -1, np.int64)
        h2 = np.zeros((T, D), np.float32)
        node2[pos2] = Xs
        h2[pos2] = Hs
        node2[start2[1:][odd] - 1] = Xs[runstarts[odd]]

        # cut the padded stream at run boundaries into 128 streams
        tgt = np.arange(1, PARTS) * ((T + PARTS - 1) // PARTS)
        ci = np.minimum(np.searchsorted(start2[:-1], tgt), R - 1)
        cuts = np.concatenate([[0], start2[:-1][ci], [T]])
        cnt = np.diff(cuts)
        assert cnt.max() <= SLOTS, f"partition stream overflow: {cnt.max()}"

        node_pad = np.full((PARTS, SLOTS), -1, np.int64)
        h_pad = np.zeros((PARTS, SLOTS, D), np.float32)
        pidx = np.repeat(np.arange(PARTS), cnt)
        eidx = np.arange(T) - np.repeat(cuts[:-1], cnt)
        node_pad[pidx, eidx] = node2
        h_pad[pidx, eidx] = h2
        m = np.zeros((PARTS, SLOTS), np.float32)
        m[:, 1:] = node_pad[:, 1:] == node_pad[:, :-1]
        m2 = m[:, 0::2]                       # pair-level mask [PARTS, 1280]

        # device layout: [PARTS, piece, d, A(640) | B(640)]
        A = h_pad[:, 0::2, :].reshape(PARTS, PIECES, HLEN, D)
        B = h_pad[:, 1::2, :].reshape(PARTS, PIECES, HLEN, D)
        dev = np.empty((PARTS, PIECES, D, PLEN), np.float32)
        dev[:, :, :, :HLEN] = A.transpose(0, 1, 3, 2)
        dev[:, :, :, HLEN:] = B.transpose(0, 1, 3, 2)
        h_dev = dev.reshape(PARTS, FREE).astype(ml_dtypes.bfloat16)
        m_dev = np.ascontiguousarray(m2).astype(ml_dtypes.bfloat16)
        in_maps.append({"h": np.ascontiguousarray(h_dev), "m": m_dev})
        metas.append(node_pad[:, 0::2])       # node id per pair

    _prog_cache["last_inputs"] = in_maps
    # The very first execution of a freshly loaded program has been
    # observed (once) to return corrupted results; correct runs are
    # bit-identical. Run until two consecutive executions agree.
    res = run_bass_kernel_spmd(nc, in_maps, core_ids=list(range(CORES)),
                               trace=False)
    for _ in range(3):
        res2 = run_bass_kernel_spmd(nc, in_maps, core_ids=list(range(CORES)),
                                    trace=False)
        if all(
            np.array_equal(
                res.results[c]["s"].view(np.uint16),
                res2.results[c]["s"].view(np.uint16))
            for c in range(CORES)
        ):
            break
        res = res2

    out = np.zeros((N, D), np.float32)
    NP = PIECES * HLEN
    for c in range(CORES):
        node_pair = metas[c]                  # [PARTS, 1280]
        s = np.asarray(res.results[c]["s"]).astype(np.float32)
        s = s.reshape(PARTS, PIECES, D, HLEN)
        nxt = np.concatenate(
            [node_pair[:, 1:], np.full((PARTS, 1), -2, np.int64)], axis=1)
        is_end = (node_pair >= 0) & (node_pair != nxt)
        pp, ii = np.nonzero(is_end)
        nodes = node_pair[pp, ii]
        vals = s[pp, ii // HLEN, :, ii % HLEN]
        # within one core each node has exactly one run end -> unique idx
        out[nodes] += vals
    return out
